# revision 54
# baseline (speedup 1.0000x reference)
"""DiT attention block on 8 Trainium2 NeuronCores.

Sharding: batch (2) x head-groups (4 heads each) -> 8 cores.  Each core
computes q/k/v projections, RMSNorm+rope on q/k, softmax attention, and
its partial output projection for its 4 heads; the host sums the 4
head-group bf16 partials per batch and transposes back.

Implementation notes (v4, all matmuls fp16/bf16 -- fp8 was measured and
rejected: its ~3% weight noise lands unaveraged on near-one-hot softmax
columns and blows the 2e-2 gate):
  - Startup: every bulk tensor is halved across the three DMA queues
    (sync/scalar HW-DGE + gpsimd SW-DGE) in first-needed-first ring
    order, so the first V matmul fires at ~20us instead of ~35us.  x
    chunks are half-major [P, 2, KO, SC2] so each half lands as one
    contiguous 8KB/partition transfer (a strided half fragments into
    512B packets at ~50GB/s).  Chunk halves for sc>=1 ride scalar+sync;
    gpsimd's SW-DGE prep would execute late behind its stage-A adds.
  - Stage A streams x in 4 chunks; K/Q are produced transposed per head
    ([head_dim, seq]); RMSNorm sum-of-squares is ONE ones[128,128]
    matmul per head that reduces AND broadcasts across partitions
    (saves the old separate per-head rstd-broadcast matmul), rstd =
    exp(-ln(.)/2) on ACT, rope rotate-half is a partition-offset
    SBUF->SBUF DMA.  Per-head tails are software-pipelined ~2 heads
    late; the last chunk's Q tails do their rope add on DVE because the
    stage-boundary drain serializes on GpSimd's 1.15us adds otherwise.
  - Stage B processes seq in halves: per (head, kt) fp16 scores ->
    exp(scale*s - 2) into bf16 "at" tiles; A@V accumulates in PSUM;
    rowsums accumulate on DVE (12/16, incl. the late kts) + GpSimd
    (only kts 1/4/7/10 -- its 2.1us adds must finish before the next
    head's rowsum matmuls read acc1).  The normalize chain is split and
    staggered into the next head's slots: rowsum matmuls at kt0/kt4,
    Ln+Exp+mul at kt2/kt6, so the Ln never head-of-line-blocks Scalar's
    exp FIFO (that convoy stalled psc recycling 2.6us/head and
    re-throttled HAM to 1.2GHz).  The single-buffer pav1 bank is
    evacuated to SBUF (bf16) by a Vector copy at kt0 so the next head's
    first A@V write never waits on the normalize chain.  Output-
    projection dt-groups from the previous half interleave at kts
    0,3,5,8,9,11,13,15 -- the kt0 group is Scalar-independent filler
    that paces psc issue against the head-start exp backlog; the rest
    stay clear of the prs bank-rotation slots.
  - Known ceilings: stage B is co-limited by Scalar (32 exps + 4 norm
    ACTs ~= 21us/head vs PE 21.3us/head); the ~10us NEFF teardown and
    ~9us DMA-ring spin-up at start are fixed costs.
"""

import math

import ml_dtypes
import numpy as np

import concourse.bass as bass
import concourse.mybir as mybir
import concourse.tile as tile
from concourse.bass_utils import run_bass_kernel_spmd

F32 = mybir.dt.float32
F16 = mybir.dt.float16
BF16 = mybir.dt.bfloat16
F8 = mybir.dt.float8e4
F8E5 = mybir.dt.float8e5
E4 = ml_dtypes.float8_e4m3
DR = mybir.MatmulPerfMode.DoubleRow
Act = mybir.ActivationFunctionType

P = 128          # partitions / head_dim
S = 2048         # sequence
D = 2048         # model dim
HD = 128         # head dim
NH = 16          # total heads
NHL = 4          # heads per core
IL = NHL * HD    # 512, inner slice per core
KO = D // P      # 16 contraction subtiles
KO2 = KO // 2    # 8 DoubleRow pairs
SC = 512         # x-chunk columns (projection phase)
SC2 = SC // 2    # token-half split of a chunk's DMA
NSC = S // SC    # 4
QC = 512         # q-chunk columns (attention phase)
NQC = S // QC    # 4
EPS = 1e-6
SCALE = 1.0 / math.sqrt(HD)
WS = math.sqrt(D)              # host-side weight pre-scale
SSQ_SCALE = 1.0 / HD          # with WS^2*EPS bias: rstd comes out as rstd_true/WS
EXP_BIAS = -2.0   # cancels in softmax; keeps exp moderate
N_CORES = 8

_PROG_CACHE = {}


def _split_multi_waits(nc, max_waits=1):
    """walrus here rejects >1 sync-wait per instruction; move extras onto
    same-engine nops placed immediately before the instruction."""
    n_split = 0
    for fn in nc.m.functions:
        for bb in fn.blocks:
            insts = bb.instructions
            new_list = []
            changed = False
            for inst in insts:
                si = getattr(inst, "sync_info", None)
                waits = list(si.on_wait) if (si is not None and si.on_wait) else []
                if len(waits) > max_waits:
                    extra = waits[:-max_waits]
                    keep = waits[-max_waits:]
                    for i in range(0, len(extra), max_waits):
                        nop = mybir.InstNoOp(
                            name=f"I-wsplit-{nc.next_id()}", ins=[], outs=[])
                        nop.engine = inst.engine
                        nop.sync_info = mybir.SyncInfo(
                            on_wait=extra[i:i + max_waits], on_update=[])
                        new_list.append(nop)
                        n_split += 1
                    del si.on_wait[:]
                    si.on_wait.extend(keep)
                    changed = True
                new_list.append(inst)
            if changed:
                del insts[:]
                insts.extend(new_list)
    return n_split


def _emit(nc, tc, t, phases=(1, 2)):
    from contextlib import ExitStack

    with ExitStack() as top:
        top.enter_context(nc.allow_low_precision(
            reason="fp8 DoubleRow matmuls; fp32 kept where it matters"))
        const = top.enter_context(tc.tile_pool(name="const", bufs=1))

        rope = {}
        for nm in ("ropeAq", "ropeBq", "ropeAk", "ropeBk"):
            til = const.tile([P, S], F16, tag=nm, name=nm)
            rope[nm] = til
        onesb = const.tile([P, P], BF16, tag="onesb")
        nc.vector.memset(onesb, 1.0)
        eps_t = const.tile([P, 1], F32, tag="eps")
        nc.vector.memset(eps_t, EPS)
        ebias = const.tile([P, 1], F32, tag="ebias")
        nc.vector.memset(ebias, EXP_BIAS)

        resid = top.enter_context(tc.tile_pool(name="resid", bufs=1))
        w8 = {}
        for nm in ("wv16", "wk16", "wq16"):
            w8[nm] = resid.tile([P, KO, IL], F16, tag=nm, name=nm)
        # DMA ring order is first-needed-first; each weight is halved
        # across the two bulk queues (scalar HW-DGE + gpsimd SW-DGE) so
        # the V projection can start ~12us earlier.  sync carries the
        # chunk-0 x halves then stays latency-clean for psw swaps.
        # wv is split 9/7: gpsimd's ring starts ~1us later and runs
        # ~10% slower, so equalizing completion gates the first matmul
        # ~2.5us earlier than an even split
        nc.scalar.dma_start(w8["wv16"][:, 0:9, :], t["wv16"][:, 0:9, :])
        nc.gpsimd.dma_start(w8["wv16"][:, 9:KO, :], t["wv16"][:, 9:KO, :])
        for nm in ("wk16", "wq16"):
            nc.scalar.dma_start(w8[nm][:, 0:KO2, :], t[nm][:, 0:KO2, :])
            nc.gpsimd.dma_start(w8[nm][:, KO2:KO, :], t[nm][:, KO2:KO, :])
        nc.scalar.dma_start(rope["ropeAq"][:], t["ropeAq"][:, :])
        nc.scalar.dma_start(rope["ropeBq"][:], t["ropeBq"][:, :])

        qres = [resid.tile([P, S], F16, tag=f"qres{h}", name=f"qres{h}")
                for h in range(NHL)]
        kres = [resid.tile([P, S], F16, tag=f"kres{h}", name=f"kres{h}")
                for h in range(NHL)]
        vres8 = resid.tile([P, KO, IL], BF16, tag="vres8")

        # ---------------- stage A: projections + RMSNorm + rope -------------
        if 1 in phases:
          with ExitStack() as ph:
            sa = ph.enter_context(tc.tile_pool(name="sa", bufs=3))
            xqp = ph.enter_context(tc.tile_pool(name="xqp", bufs=2))
            nrm = ph.enter_context(tc.tile_pool(name="nrm", bufs=2))
            psA = ph.enter_context(tc.tile_pool(name="psA", bufs=1, space="PSUM"))

            from collections import deque
            pend = deque()      # deferred small PE-tail closures

            def pop_pend(n, lag=2):
                # keep `lag` closures queued so tails run ~2 units late
                for _ in range(n):
                    if len(pend) > lag:
                        pend.popleft()()

            def emit_head(wname, ra, rb, dst, s0, xq, h, tail_eng=None):
                """fp16 projection for one head; defer ssq/psw/rope tail."""
                praw = psA.tile([P, SC], F32, tag="praw", bufs=2,
                                name="praw")
                for kk in range(KO):
                    nc.tensor.matmul(
                        praw[:, :],
                        lhsT=w8[wname][:, kk, h * P:(h + 1) * P],
                        rhs=xq[:, :, kk, :],
                        start=(kk == 0), stop=(kk == KO - 1))
                raw = sa.tile([P, SC], F16, tag="raw", name="raw")
                nc.scalar.copy(raw[:], praw[:, :])
                sq = sa.tile([P, SC], BF16, tag="sq", name="sq")
                nc.scalar.activation(sq[:], praw[:, :], func=Act.Square)

                def tail():
                    # sum-of-squares reduced AND broadcast across all
                    # 128 partitions in one ones[128,128] matmul (the
                    # old [1,SC] reduce needed a second broadcast
                    # matmul per head; this saves 512 PE cols/head)
                    pbq = psA.tile([P, SC], F32, tag="pssq", bufs=4,
                                   name="pbq")
                    nc.tensor.matmul(pbq[:, :], lhsT=onesb[:, :],
                                     rhs=sq[:], start=True, stop=True)
                    psw = sa.tile([P, SC], F16, tag="psw", name="psw")
                    nc.sync.dma_start(psw[0:64, :], raw[64:128, :])
                    nc.sync.dma_start(psw[64:128, :], raw[0:64, :])
                    tmp = sa.tile([P, SC], F16, tag="tmp", name="tmp")
                    nc.vector.tensor_mul(tmp[:], ra[:, s0:s0 + SC], raw[:])
                    tmp2 = sa.tile([P, SC], F16, tag="tmp2", name="tmp2")
                    nc.vector.tensor_mul(tmp2[:], rb[:, s0:s0 + SC],
                                         psw[:, :])
                    t3 = sa.tile([P, SC], F16, tag="t3", name=f"t3_{h}",
                                 bufs=6)
                    (tail_eng or nc.gpsimd).tensor_add(t3[:], tmp[:],
                                                       tmp2[:])

                    def tail2():
                        lnv = nrm.tile([P, SC], F32, tag="lnv",
                                       name="lnv")
                        nc.scalar.activation(lnv[:], pbq[:, :],
                                             func=Act.Ln, bias=eps_t[:],
                                             scale=SSQ_SCALE)
                        rstd = nrm.tile([P, SC], F16, tag="rstd",
                                        name="rstd")
                        nc.scalar.activation(rstd[:], lnv[:],
                                             func=Act.Exp, scale=-0.5)
                        nc.vector.tensor_mul(
                            dst[h][:, s0:s0 + SC], t3[:], rstd[:])
                    pend.append(tail2)
                pend.append(tail)

            for sc in range(NSC):
                s0 = sc * SC
                # V projection ([s, il] layout; x subtiles stationary).
                # xq is half-major [P, 2, KO, SC2] so each DMA half is a
                # contiguous 8KB/partition transfer (strided halves
                # fragmented into 512B packets and ran at ~50GB/s).
                xq = xqp.tile([P, 2, KO, SC2], F16, tag="xq", name="xq")
                if sc == 0:
                    nc.sync.dma_start(xq[:, 0], t["x16"][sc, 0, :, :, :])
                    nc.sync.dma_start(xq[:, 1], t["x16"][sc, 1, :, :, :])
                    # K-rope consts ride sync after the chunk-0 halves
                    nc.sync.dma_start(rope["ropeAk"][:], t["ropeAk"][:, :])
                    nc.sync.dma_start(rope["ropeBk"][:], t["ropeBk"][:, :])
                else:
                    nc.scalar.dma_start(xq[:, 0], t["x16"][sc, 0, :, :, :])
                    nc.sync.dma_start(xq[:, 1], t["x16"][sc, 1, :, :, :])
                for st in range(SC // P):
                    hf, ho = st // 2, (st % 2) * P
                    pv = psA.tile([P, SC], F32, tag="pv", bufs=2, name="pv")
                    for kk in range(KO):
                        nc.tensor.matmul(
                            pv[:, :],
                            lhsT=xq[:, hf, kk, ho:ho + P],
                            rhs=w8["wv16"][:, kk, :],
                            start=(kk == 0), stop=(kk == KO - 1))
                    nc.vector.tensor_copy(vres8[:, sc * (SC // P) + st, :],
                                          pv[:, :])
                    pop_pend(2)

                # K then Q projections (transposed per head) + norm + rope
                for wname, ra, rb, dst in (
                    ("wk16", rope["ropeAk"], rope["ropeBk"], kres),
                    ("wq16", rope["ropeAq"], rope["ropeBq"], qres),
                ):
                    # last chunk's Q tails drain serially at the stage
                    # boundary; Vector's 426ns adds beat GpSimd's 1.15us
                    teng = (nc.vector if (sc == NSC - 1
                                          and wname == "wq16") else None)
                    for h in range(NHL):
                        emit_head(wname, ra, rb, dst, s0, xq, h, teng)
                        pop_pend(2)
            while pend:
                pend.popleft()()

        if "dbg_qkv" in t:
            for h in range(NHL):
                nc.sync.dma_start(t["dbg_qkv"][0, h, :, :], qres[h][:])
                nc.sync.dma_start(t["dbg_qkv"][1, h, :, :], kres[h][:])
        if "dbg_v" in t:
            nc.sync.dma_start(t["dbg_v"][:, :, :], vres8[:])

        # ---------------- stage B: attention + output projection ------------
        if 2 in phases:
          with ExitStack() as ph:
            atp = ph.enter_context(tc.tile_pool(name="atp", bufs=5))
            wop = ph.enter_context(tc.tile_pool(name="wop", bufs=1))
            wo = wop.tile([P, NHL, D], BF16, tag="wo", name="wo")
            nc.gpsimd.dma_start(wo[:], t["woT"][:, :, :])
            avp = ph.enter_context(tc.tile_pool(name="avp", bufs=5))
            rcpp = ph.enter_context(tc.tile_pool(name="rcpp", bufs=2))
            outs = ph.enter_context(tc.tile_pool(name="outs", bufs=4))
            psB = ph.enter_context(tc.tile_pool(name="psB", bufs=1, space="PSUM"))

            from collections import deque
            pend_o = deque()    # out-proj dt-group closures from prev qc

            def emit_oproj(avn, q0, dt):
                po = psB.tile([P, QC], F32, tag="po", bufs=2, name="po")
                for it in range(NHL):
                    nc.tensor.matmul(
                        po[:, :],
                        lhsT=wo[:, it, dt * P:(dt + 1) * P],
                        rhs=avn[it][:],
                        start=(it == 0), stop=(it == NHL - 1))
                osb = outs.tile([P, QC], BF16, tag="osb", name="osb")
                nc.vector.tensor_copy(osb[:], po[:, :])
                nc.sync.dma_start(
                    t["outT"][dt * P:(dt + 1) * P, q0:q0 + QC], osb[:])

            SH = 2 * QC          # q-half width (1024)
            accp = ph.enter_context(tc.tile_pool(name="accp", bufs=3))
            pend_nm = deque()    # deferred rowsum matmuls (per head)
            pend_na = deque()    # deferred normalize ACT chains (per head)

            def make_norm_mm(acc0, acc1, prs_store, jq, pav=None):
                # rowsum matmuls run ~2 slots ahead of the ACT chain so
                # the Ln never waits on them inside Scalar's FIFO
                # (head-of-line blocking there stalled psc recycling).
                # jq1 first: its chain releases the single-buffer pav1.
                def norm_mm():
                    if pav is not None:
                        # evacuate single-buffer pav1 to SBUF right at
                        # kt0 so the next head's first A@V write never
                        # waits for the normalize chain (which sits
                        # behind Scalar's head-start exp backlog)
                        pavs = avp.tile([P, QC], BF16, tag="pavs",
                                        name="pavs", bufs=2)
                        nc.vector.tensor_copy(pavs[:], pav[jq][:, :])
                        prs_store["pavs"] = pavs
                    prs = psB.tile([P, QC], F32, tag="po", bufs=2,
                                   name="prs")
                    nc.tensor.matmul(
                        prs[:, :], lhsT=onesb[:, :],
                        rhs=acc0[:, jq * QC:(jq + 1) * QC],
                        start=True, stop=False)
                    nc.tensor.matmul(
                        prs[:, :], lhsT=onesb[:, :],
                        rhs=acc1[:, jq * QC:(jq + 1) * QC],
                        start=False, stop=True)
                    prs_store[jq] = prs
                return norm_mm

            def make_norm_act(h, pav, prs_store, avn, sh_done, jq):
                def norm_act():
                    lnp = rcpp.tile([P, QC], F32, tag="lnp", name="lnp")
                    nc.scalar.activation(lnp[:], prs_store[jq][:, :],
                                         func=Act.Ln)
                    rcpb = rcpp.tile([P, QC], BF16, tag="rcpb",
                                     name="rcpb")
                    nc.scalar.activation(rcpb[:], lnp[:],
                                         func=Act.Exp, scale=-1.0)
                    an = avp.tile([P, QC], BF16, tag="avn",
                                  name=f"an{h}_{jq}", bufs=10)
                    src = (prs_store["pavs"] if "pavs" in prs_store
                           and jq == 1 else pav[jq])
                    nc.vector.tensor_mul(an[:], src[:, :], rcpb[:])
                    avn[(h, jq)] = an
                    if sh_done is not None:
                        sh_done()
                return norm_act

            for sh in range(S // SH):
                q0 = sh * SH
                avn = {}         # (h, jq) -> normalized attention out

                def make_sh_done(avn=avn, q0=q0):
                    def sh_done():
                        if "dbg_avn" in t and q0 == 0:
                            for hh in range(NHL):
                                nc.sync.dma_start(t["dbg_avn"][hh, :, :],
                                                  avn[(hh, 0)][:])
                        for jq in (0, 1):
                            alist = [avn[(it, jq)] for it in range(NHL)]
                            for dt in range(D // P):
                                pend_o.append(
                                    lambda avn=alist, qq=q0 + jq * QC,
                                    dt=dt: emit_oproj(avn, qq, dt))
                    return sh_done

                for h in range(NHL):
                    pav0 = psB.tile([P, QC], F32, tag="pav0", bufs=2,
                                    name="pav0")
                    pav1 = psB.tile([P, QC], F32, tag="pav1", bufs=1,
                                    name="pav1")
                    pav = (pav0, pav1)
                    acc0 = accp.tile([P, SH], BF16, tag="acc0", name="acc0")
                    acc1 = accp.tile([P, SH], BF16, tag="acc1", name="acc1")
                    atq = deque()

                    def emit_av(h=h, pav=pav, acc0=acc0, acc1=acc1):
                        kt, a8 = atq.popleft()
                        for jq in (0, 1):
                            nc.tensor.matmul(
                                pav[jq][:, :],
                                lhsT=vres8[:, kt, h * HD:(h + 1) * HD],
                                rhs=a8[:, jq * QC:(jq + 1) * QC],
                                start=(kt == 0), stop=(kt == KO - 1))
                        # rowsum accumulators: Pool takes 4 early/mid
                        # tiles only -- its 2.1us adds must all land
                        # before the next head's prs matmuls read acc1
                        if kt in (1, 4, 7, 10):
                            eng, acct = nc.gpsimd, acc1
                            first = kt == 1
                        else:
                            eng, acct = nc.vector, acc0
                            first = kt == 0
                        if first:
                            eng.tensor_copy(acct[:], a8[:])
                        else:
                            eng.tensor_add(acct[:], acct[:], a8[:])

                    for kt in range(KO):
                        at8 = atp.tile([P, SH], BF16, tag="at8",
                                       name="at8")
                        for jq in (0, 1):
                            psc = psB.tile([P, QC], F32, tag="psc",
                                           bufs=3, name="psc")
                            nc.tensor.matmul(
                                psc[:, :],
                                lhsT=kres[h][:, kt * P:(kt + 1) * P],
                                rhs=qres[h][:, q0 + jq * QC:
                                            q0 + (jq + 1) * QC],
                                start=True, stop=True)
                            nc.scalar.activation(
                                at8[:, jq * QC:(jq + 1) * QC], psc[:, :],
                                func=Act.Exp, bias=ebias[:], scale=SCALE)
                        atq.append((kt, at8))
                        if len(atq) >= 3:
                            emit_av()
                        if kt in (0, 4) and pend_nm:
                            pend_nm.popleft()()
                        if kt in (2, 6) and pend_na:
                            pend_na.popleft()()
                        # the kt0 po-group is Scalar-independent PE
                        # filler that paces psc issue while Scalar
                        # drains its head-start exp backlog
                        if pend_o and kt in (0, 3, 5, 8, 9, 11,
                                             13, 15):
                            pend_o.popleft()()
                    while atq:
                        emit_av()
                    done = make_sh_done() if h == NHL - 1 else None
                    prs_store = {}
                    pend_nm.append(make_norm_mm(acc0, acc1, prs_store, 1,
                                                pav))
                    pend_nm.append(make_norm_mm(acc0, acc1, prs_store, 0))
                    pend_na.append(make_norm_act(h, pav, prs_store, avn,
                                                 None, 1))
                    pend_na.append(make_norm_act(h, pav, prs_store, avn,
                                                 done, 0))
            while pend_nm:
                pend_nm.popleft()()
            while pend_na:
                pend_na.popleft()()
            while pend_o:
                pend_o.popleft()()


def _build_program(loop_n=0, phases=(1, 2)):
    key = ("nc", loop_n, tuple(phases))
    if key in _PROG_CACHE:
        return _PROG_CACHE[key]
    nc = bass.Bass()
    t = {}
    t["wq16"] = nc.dram_tensor("wq16", [P, KO, IL], F16, kind="ExternalInput")
    t["wk16"] = nc.dram_tensor("wk16", [P, KO, IL], F16, kind="ExternalInput")
    t["wv16"] = nc.dram_tensor("wv16", [P, KO, IL], F16, kind="ExternalInput")
    t["x16"] = nc.dram_tensor("x16", [NSC, 2, P, KO, SC2], F16,
                              kind="ExternalInput")
    t["woT"] = nc.dram_tensor("woT", [P, NHL, D], BF16, kind="ExternalInput")
    for nm in ("ropeAq", "ropeBq", "ropeAk", "ropeBk"):
        t[nm] = nc.dram_tensor(nm, [P, S], F16, kind="ExternalInput")
    t["outT"] = nc.dram_tensor("outT", [D, S], BF16, kind="ExternalOutput")
    if loop_n == -1:  # debug taps build
        t["dbg_qkv"] = nc.dram_tensor("dbg_qkv", [2, NHL, P, S], F16,
                                      kind="ExternalOutput")
        t["dbg_v"] = nc.dram_tensor("dbg_v", [P, KO, IL], BF16,
                                    kind="ExternalOutput")
        t["dbg_avn"] = nc.dram_tensor("dbg_avn", [NHL, P, QC], BF16,
                                      kind="ExternalOutput")

    with tile.TileContext(nc) as tc:
        _emit(nc, tc, t, phases)
    _split_multi_waits(nc)
    _PROG_CACHE[key] = nc
    return nc


def _prep_in_maps(x, rope_emb, Wq, Wk, Wv, Wo, q_norm_w, k_norm_w):
    x = np.asarray(x, np.float32)
    F = np.asarray(rope_emb, np.float32)[:, 0]          # [S, 64, 2, 2]
    A0 = np.concatenate([F[:, :, 0, 0], F[:, :, 1, 1]], axis=-1)  # [S, 128]
    B0 = np.concatenate([F[:, :, 0, 1], F[:, :, 1, 0]], axis=-1)  # [S, 128]

    def rope_consts(w):
        w = np.asarray(w, np.float32)
        w_sw = np.concatenate([w[64:], w[:64]])
        A = np.ascontiguousarray((A0 * w[None, :]).T).astype(np.float16)
        B = np.ascontiguousarray((B0 * w_sw[None, :]).T).astype(np.float16)
        return A, B

    Aq, Bq = rope_consts(q_norm_w)
    Ak, Bk = rope_consts(k_norm_w)

    def to_dev(arr, kt, width, dtype):
        # [D_like, width] -> [128, kt, width] with row index = kt*128 + p
        return np.ascontiguousarray(
            arr.reshape(kt, P, width).transpose(1, 0, 2)).astype(dtype)

    Wq = np.asarray(Wq, np.float32)
    Wk = np.asarray(Wk, np.float32)
    Wv = np.asarray(Wv, np.float32)
    Wo = np.asarray(Wo, np.float32)

    in_maps = []
    for c in range(N_CORES):
        b, hg = divmod(c, NH // NHL)
        sl = slice(hg * IL, (hg + 1) * IL)
        in_maps.append({
            "wq16": to_dev(np.ascontiguousarray(Wq[sl, :].T), KO, IL,
                           np.float16),
            "wk16": to_dev(np.ascontiguousarray(Wk[sl, :].T), KO, IL,
                           np.float16),
            "wv16": to_dev(np.ascontiguousarray(Wv[sl, :].T), KO, IL,
                           np.float16),
            "x16": np.ascontiguousarray(
                x[b].T.reshape(KO, P, NSC, 2, SC2).transpose(2, 3, 1, 0, 4)
            ).astype(np.float16),
            "woT": to_dev(np.ascontiguousarray(Wo[:, sl].T), NHL, D,
                          ml_dtypes.bfloat16),
            "ropeAq": Aq, "ropeBq": Bq, "ropeAk": Ak, "ropeBk": Bk,
        })
    return in_maps


def kernel(x, rope_emb, Wq, Wk, Wv, Wo, q_norm_w, k_norm_w, _trace=False):
    nc = _build_program()
    in_maps = _prep_in_maps(x, rope_emb, Wq, Wk, Wv, Wo, q_norm_w, k_norm_w)
    res = run_bass_kernel_spmd(nc, in_maps, core_ids=list(range(N_CORES)),
                               trace=_trace)
    out = np.empty((2, S, D), np.float32)
    for b in range(2):
        acc = res.results[4 * b]["outT"].astype(np.float32)
        for hg in range(1, 4):
            acc += res.results[4 * b + hg]["outT"].astype(np.float32)
        out[b] = acc.T
    if _trace:
        kernel.last_exec_time_ns = res.exec_time_ns
        kernel.last_results = res
    return out



# revision 60
# speedup vs baseline: 1.0126x; 1.0126x over previous
"""DiT attention block on 8 Trainium2 NeuronCores.

Sharding: batch (2) x head-groups (4 heads each) -> 8 cores.  Each core
computes q/k/v projections, RMSNorm+rope on q/k, softmax attention, and
its partial output projection for its 4 heads; the host sums the 4
head-group bf16 partials per batch and transposes back.

Implementation notes (v4, all matmuls fp16/bf16 -- fp8 was measured and
rejected: its ~3% weight noise lands unaveraged on near-one-hot softmax
columns and blows the 2e-2 gate):
  - Startup: every bulk tensor is halved across the three DMA queues
    (sync/scalar HW-DGE + gpsimd SW-DGE) in first-needed-first ring
    order, so the first V matmul fires at ~20us instead of ~35us.  x
    chunks are half-major [P, 2, KO, SC2] so each half lands as one
    contiguous 8KB/partition transfer (a strided half fragments into
    512B packets at ~50GB/s).  Chunk halves for sc>=1 ride scalar+sync;
    gpsimd's SW-DGE prep would execute late behind its stage-A adds.
  - Stage A streams x in 4 chunks; K/Q are produced transposed per head
    ([head_dim, seq]); RMSNorm sum-of-squares is ONE ones[128,128]
    matmul per head that reduces AND broadcasts across partitions
    (saves the old separate per-head rstd-broadcast matmul), rstd =
    exp(-ln(.)/2) on ACT, rope rotate-half is a partition-offset
    SBUF->SBUF DMA.  Per-head tails are software-pipelined ~2 heads
    late; the last chunk's Q tails do their rope add on DVE because the
    stage-boundary drain serializes on GpSimd's 1.15us adds otherwise.
  - Stage B processes seq in halves: per (head, kt) fp16 scores ->
    exp(scale*s - 2) into bf16 "at" tiles; A@V accumulates in PSUM;
    rowsums accumulate on DVE (12/16, incl. the late kts) + GpSimd
    (only kts 1/4/7/10 -- its 2.1us adds must finish before the next
    head's rowsum matmuls read acc1).  The normalize chain is split and
    staggered into the next head's slots: rowsum matmuls at kt0/kt4,
    Ln+Exp+mul at kt2/kt6, so the Ln never head-of-line-blocks Scalar's
    exp FIFO (that convoy stalled psc recycling 2.6us/head and
    re-throttled HAM to 1.2GHz).  The single-buffer pav1 bank is
    evacuated to SBUF (bf16) by a Vector copy at kt0 so the next head's
    first A@V write never waits on the normalize chain.  Output-
    projection dt-groups from the previous half interleave at kts
    0,3,5,8,9,11,13,15 -- the kt0 group is Scalar-independent filler
    that paces psc issue against the head-start exp backlog; the rest
    stay clear of the prs bank-rotation slots.
  - Known ceilings: stage B is co-limited by Scalar (32 exps + 4 norm
    ACTs ~= 21us/head vs PE 21.3us/head); the ~10us NEFF teardown and
    ~9us DMA-ring spin-up at start are fixed costs.
"""

import math

import ml_dtypes
import numpy as np

import concourse.bass as bass
import concourse.mybir as mybir
import concourse.tile as tile
from concourse.bass_utils import run_bass_kernel_spmd

F32 = mybir.dt.float32
F16 = mybir.dt.float16
BF16 = mybir.dt.bfloat16
F8 = mybir.dt.float8e4
F8E5 = mybir.dt.float8e5
E4 = ml_dtypes.float8_e4m3
DR = mybir.MatmulPerfMode.DoubleRow
Act = mybir.ActivationFunctionType

P = 128          # partitions / head_dim
S = 2048         # sequence
D = 2048         # model dim
HD = 128         # head dim
NH = 16          # total heads
NHL = 4          # heads per core
IL = NHL * HD    # 512, inner slice per core
KO = D // P      # 16 contraction subtiles
KO2 = KO // 2    # 8 DoubleRow pairs
SC = 512         # x-chunk columns (projection phase)
SC2 = SC // 2    # token-half split of a chunk's DMA
NSC = S // SC    # 4
QC = 512         # q-chunk columns (attention phase)
NQC = S // QC    # 4
EPS = 1e-6
SCALE = 1.0 / math.sqrt(HD)
WS = math.sqrt(D)              # host-side weight pre-scale
SSQ_SCALE = 1.0 / HD          # with WS^2*EPS bias: rstd comes out as rstd_true/WS
EXP_BIAS = -2.0   # cancels in softmax; keeps exp moderate
N_CORES = 8

_PROG_CACHE = {}


def _split_multi_waits(nc, max_waits=1):
    """walrus here rejects >1 sync-wait per instruction; move extras onto
    same-engine nops placed immediately before the instruction."""
    n_split = 0
    for fn in nc.m.functions:
        for bb in fn.blocks:
            insts = bb.instructions
            new_list = []
            changed = False
            for inst in insts:
                si = getattr(inst, "sync_info", None)
                waits = list(si.on_wait) if (si is not None and si.on_wait) else []
                if len(waits) > max_waits:
                    extra = waits[:-max_waits]
                    keep = waits[-max_waits:]
                    for i in range(0, len(extra), max_waits):
                        nop = mybir.InstNoOp(
                            name=f"I-wsplit-{nc.next_id()}", ins=[], outs=[])
                        nop.engine = inst.engine
                        nop.sync_info = mybir.SyncInfo(
                            on_wait=extra[i:i + max_waits], on_update=[])
                        new_list.append(nop)
                        n_split += 1
                    del si.on_wait[:]
                    si.on_wait.extend(keep)
                    changed = True
                new_list.append(inst)
            if changed:
                del insts[:]
                insts.extend(new_list)
    return n_split


def _emit(nc, tc, t, phases=(1, 2)):
    from contextlib import ExitStack

    with ExitStack() as top:
        top.enter_context(nc.allow_low_precision(
            reason="fp8 DoubleRow matmuls; fp32 kept where it matters"))
        const = top.enter_context(tc.tile_pool(name="const", bufs=1))

        rope = {}
        for nm in ("ropeAq", "ropeBq", "ropeAk", "ropeBk"):
            til = const.tile([P, S], F16, tag=nm, name=nm)
            rope[nm] = til
        onesb = const.tile([P, P], BF16, tag="onesb")
        nc.vector.memset(onesb, 1.0)
        eps_t = const.tile([P, 1], F32, tag="eps")
        nc.vector.memset(eps_t, EPS)
        ebias = const.tile([P, 1], F32, tag="ebias")
        nc.vector.memset(ebias, EXP_BIAS)

        resid = top.enter_context(tc.tile_pool(name="resid", bufs=1))
        w8 = {}
        for nm in ("wv16", "wk16", "wq16"):
            w8[nm] = resid.tile([P, KO, IL], F16, tag=nm, name=nm)
        # DMA ring order is first-needed-first; each weight is halved
        # across the two bulk queues (scalar HW-DGE + gpsimd SW-DGE) so
        # the V projection can start ~12us earlier.  sync carries the
        # chunk-0 x halves then stays latency-clean for psw swaps.
        # wv is split 9/7: gpsimd's ring starts ~1us later and runs
        # ~10% slower, so equalizing completion gates the first matmul
        # ~2.5us earlier than an even split
        nc.scalar.dma_start(w8["wv16"][:, 0:9, :], t["wv16"][:, 0:9, :])
        nc.gpsimd.dma_start(w8["wv16"][:, 9:KO, :], t["wv16"][:, 9:KO, :])
        for nm in ("wk16", "wq16"):
            nc.scalar.dma_start(w8[nm][:, 0:KO2, :], t[nm][:, 0:KO2, :])
            nc.gpsimd.dma_start(w8[nm][:, KO2:KO, :], t[nm][:, KO2:KO, :])
        nc.scalar.dma_start(rope["ropeAq"][:], t["ropeAq"][:, :])
        nc.scalar.dma_start(rope["ropeBq"][:], t["ropeBq"][:, :])

        qres = [resid.tile([P, S], F16, tag=f"qres{h}", name=f"qres{h}")
                for h in range(NHL)]
        kres = [resid.tile([P, S], F16, tag=f"kres{h}", name=f"kres{h}")
                for h in range(NHL)]
        vres8 = resid.tile([P, KO, IL], BF16, tag="vres8")

        # ---------------- stage A: projections + RMSNorm + rope -------------
        if 1 in phases:
          with ExitStack() as ph:
            sa = ph.enter_context(tc.tile_pool(name="sa", bufs=3))
            xqp = ph.enter_context(tc.tile_pool(name="xqp", bufs=2))
            nrm = ph.enter_context(tc.tile_pool(name="nrm", bufs=2))
            psA = ph.enter_context(tc.tile_pool(name="psA", bufs=1, space="PSUM"))

            from collections import deque
            pend = deque()      # deferred small PE-tail closures

            def pop_pend(n, lag=2):
                # keep `lag` closures queued so tails run ~2 units late
                for _ in range(n):
                    if len(pend) > lag:
                        pend.popleft()()

            def emit_head(wname, ra, rb, dst, s0, xq, h, tail_eng=None):
                """fp16 projection for one head; defer ssq/psw/rope tail."""
                praw = psA.tile([P, SC], F32, tag="praw", bufs=2,
                                name="praw")
                for kk in range(KO):
                    nc.tensor.matmul(
                        praw[:, :],
                        lhsT=w8[wname][:, kk, h * P:(h + 1) * P],
                        rhs=xq[:, :, kk, :],
                        start=(kk == 0), stop=(kk == KO - 1))
                raw = sa.tile([P, SC], F16, tag="raw", name="raw")
                nc.scalar.copy(raw[:], praw[:, :])
                sq = sa.tile([P, SC], BF16, tag="sq", name="sq")
                nc.scalar.activation(sq[:], praw[:, :], func=Act.Square)

                def tail():
                    # sum-of-squares reduced AND broadcast across all
                    # 128 partitions in one ones[128,128] matmul (the
                    # old [1,SC] reduce needed a second broadcast
                    # matmul per head; this saves 512 PE cols/head)
                    pbq = psA.tile([P, SC], F32, tag="pssq", bufs=4,
                                   name="pbq")
                    nc.tensor.matmul(pbq[:, :], lhsT=onesb[:, :],
                                     rhs=sq[:], start=True, stop=True)
                    psw = sa.tile([P, SC], F16, tag="psw", name="psw")
                    nc.sync.dma_start(psw[0:64, :], raw[64:128, :])
                    nc.sync.dma_start(psw[64:128, :], raw[0:64, :])
                    tmp = sa.tile([P, SC], F16, tag="tmp", name="tmp")
                    nc.vector.tensor_mul(tmp[:], ra[:, s0:s0 + SC], raw[:])
                    tmp2 = sa.tile([P, SC], F16, tag="tmp2", name="tmp2")
                    nc.vector.tensor_mul(tmp2[:], rb[:, s0:s0 + SC],
                                         psw[:, :])
                    t3 = sa.tile([P, SC], F16, tag="t3", name=f"t3_{h}",
                                 bufs=6)
                    (tail_eng or nc.gpsimd).tensor_add(t3[:], tmp[:],
                                                       tmp2[:])

                    def tail2():
                        lnv = nrm.tile([P, SC], F32, tag="lnv",
                                       name="lnv")
                        nc.scalar.activation(lnv[:], pbq[:, :],
                                             func=Act.Ln, bias=eps_t[:],
                                             scale=SSQ_SCALE)
                        rstd = nrm.tile([P, SC], F16, tag="rstd",
                                        name="rstd")
                        nc.scalar.activation(rstd[:], lnv[:],
                                             func=Act.Exp, scale=-0.5)
                        nc.vector.tensor_mul(
                            dst[h][:, s0:s0 + SC], t3[:], rstd[:])
                    pend.append(tail2)
                pend.append(tail)

            for sc in range(NSC):
                s0 = sc * SC
                # V projection ([s, il] layout; x subtiles stationary).
                # xq is half-major [P, 2, KO, SC2] so each DMA half is a
                # contiguous 8KB/partition transfer (strided halves
                # fragmented into 512B packets and ran at ~50GB/s).
                xq = xqp.tile([P, 2, KO, SC2], F16, tag="xq", name="xq")
                if sc == 0:
                    nc.sync.dma_start(xq[:, 0], t["x16"][sc, 0, :, :, :])
                    nc.sync.dma_start(xq[:, 1], t["x16"][sc, 1, :, :, :])
                    # K-rope consts ride sync after the chunk-0 halves
                    nc.sync.dma_start(rope["ropeAk"][:], t["ropeAk"][:, :])
                    nc.sync.dma_start(rope["ropeBk"][:], t["ropeBk"][:, :])
                else:
                    nc.scalar.dma_start(xq[:, 0], t["x16"][sc, 0, :, :, :])
                    nc.sync.dma_start(xq[:, 1], t["x16"][sc, 1, :, :, :])
                for st in range(SC // P):
                    hf, ho = st // 2, (st % 2) * P
                    pv = psA.tile([P, SC], F32, tag="pv", bufs=2, name="pv")
                    for kk in range(KO):
                        nc.tensor.matmul(
                            pv[:, :],
                            lhsT=xq[:, hf, kk, ho:ho + P],
                            rhs=w8["wv16"][:, kk, :],
                            start=(kk == 0), stop=(kk == KO - 1))
                    nc.vector.tensor_copy(vres8[:, sc * (SC // P) + st, :],
                                          pv[:, :])
                    pop_pend(2)

                # K then Q projections (transposed per head) + norm + rope
                for wname, ra, rb, dst in (
                    ("wk16", rope["ropeAk"], rope["ropeBk"], kres),
                    ("wq16", rope["ropeAq"], rope["ropeBq"], qres),
                ):
                    # last chunk's Q tails drain serially at the stage
                    # boundary; Vector's 426ns adds beat GpSimd's 1.15us
                    teng = (nc.vector if (sc == NSC - 1
                                          and wname == "wq16") else None)
                    for h in range(NHL):
                        emit_head(wname, ra, rb, dst, s0, xq, h, teng)
                        pop_pend(2)
            while pend:
                pend.popleft()()

        if "dbg_qkv" in t:
            for h in range(NHL):
                nc.sync.dma_start(t["dbg_qkv"][0, h, :, :], qres[h][:])
                nc.sync.dma_start(t["dbg_qkv"][1, h, :, :], kres[h][:])
        if "dbg_v" in t:
            nc.sync.dma_start(t["dbg_v"][:, :, :], vres8[:])

        # ---------------- stage B: attention + output projection ------------
        if 2 in phases:
          with ExitStack() as ph:
            atp = ph.enter_context(tc.tile_pool(name="atp", bufs=5))
            wop = ph.enter_context(tc.tile_pool(name="wop", bufs=1))
            wo = wop.tile([P, NHL, D], BF16, tag="wo", name="wo")
            nc.gpsimd.dma_start(wo[:], t["woT"][:, :, :])
            avp = ph.enter_context(tc.tile_pool(name="avp", bufs=5))
            rcpp = ph.enter_context(tc.tile_pool(name="rcpp", bufs=2))
            outs = ph.enter_context(tc.tile_pool(name="outs", bufs=4))
            psB = ph.enter_context(tc.tile_pool(name="psB", bufs=1, space="PSUM"))

            from collections import deque
            pend_o = deque()    # out-proj dt-group closures from prev qc

            def emit_oproj(avn, q0, dt, burst=False):
                po = psB.tile([P, QC], F32, tag="po", bufs=2, name="po")
                for it in range(NHL):
                    nc.tensor.matmul(
                        po[:, :],
                        lhsT=wo[:, it, dt * P:(dt + 1) * P],
                        rhs=avn[it][:],
                        start=(it == 0), stop=(it == NHL - 1))
                osb = outs.tile([P, QC], BF16, tag="osb", name="osb")
                if burst:
                    # in the closing oproj burst Scalar has no exps
                    # left; putting the copies there frees Vector and
                    # decouples the po bank rotation from its queue
                    nc.scalar.copy(osb[:], po[:, :])
                else:
                    nc.vector.tensor_copy(osb[:], po[:, :])
                nc.sync.dma_start(
                    t["outT"][dt * P:(dt + 1) * P, q0:q0 + QC], osb[:])

            SH = 2 * QC          # q-half width (1024)
            accp = ph.enter_context(tc.tile_pool(name="accp", bufs=3))
            pend_nm = deque()    # deferred rowsum matmuls (per head)
            pend_na = deque()    # deferred normalize ACT chains (per head)

            def make_norm_mm(acc0, acc1, prs_store, jq, pav=None):
                # rowsum matmuls run ~2 slots ahead of the ACT chain so
                # the Ln never waits on them inside Scalar's FIFO
                # (head-of-line blocking there stalled psc recycling).
                # jq1 first: its chain releases the single-buffer pav1.
                def norm_mm():
                    if pav is not None:
                        # evacuate single-buffer pav1 to SBUF right at
                        # kt0 so the next head's first A@V write never
                        # waits for the normalize chain (which sits
                        # behind Scalar's head-start exp backlog)
                        pavs = avp.tile([P, QC], BF16, tag="pavs",
                                        name="pavs", bufs=2)
                        nc.vector.tensor_copy(pavs[:], pav[jq][:, :])
                        prs_store["pavs"] = pavs
                    prs = psB.tile([P, QC], F32, tag="po", bufs=2,
                                   name="prs")
                    nc.tensor.matmul(
                        prs[:, :], lhsT=onesb[:, :],
                        rhs=acc0[:, jq * QC:(jq + 1) * QC],
                        start=True, stop=False)
                    nc.tensor.matmul(
                        prs[:, :], lhsT=onesb[:, :],
                        rhs=acc1[:, jq * QC:(jq + 1) * QC],
                        start=False, stop=True)
                    prs_store[jq] = prs
                return norm_mm

            def make_norm_act(h, pav, prs_store, avn, sh_done, jq):
                def norm_act():
                    lnp = rcpp.tile([P, QC], F32, tag="lnp", name="lnp")
                    nc.scalar.activation(lnp[:], prs_store[jq][:, :],
                                         func=Act.Ln)
                    rcpb = rcpp.tile([P, QC], BF16, tag="rcpb",
                                     name="rcpb")
                    nc.scalar.activation(rcpb[:], lnp[:],
                                         func=Act.Exp, scale=-1.0)
                    an = avp.tile([P, QC], BF16, tag="avn",
                                  name=f"an{h}_{jq}", bufs=16)
                    src = (prs_store["pavs"] if "pavs" in prs_store
                           and jq == 1 else pav[jq])
                    nc.vector.tensor_mul(an[:], src[:, :], rcpb[:])
                    avn[(h, jq)] = an
                    if sh_done is not None:
                        sh_done()
                return norm_act

            for sh in range(S // SH):
                q0 = sh * SH
                avn = {}         # (h, jq) -> normalized attention out

                def make_sh_done(avn=avn, q0=q0):
                    def sh_done():
                        if "dbg_avn" in t and q0 == 0:
                            for hh in range(NHL):
                                nc.sync.dma_start(t["dbg_avn"][hh, :, :],
                                                  avn[(hh, 0)][:])
                        # jq1 first: at the final drain these groups
                        # unblock after act1 alone, so they overlap the
                        # jq0 normalize chains
                        burst = q0 == S - SH
                        for jq in (1, 0):
                            for dt in range(D // P):
                                # avn looked up lazily: jq0 tiles don't
                                # exist yet when sh_done fires on act1
                                pend_o.append(
                                    lambda jq=jq, qq=q0 + jq * QC,
                                    dt=dt, b=burst: emit_oproj(
                                        [avn[(it, jq)]
                                         for it in range(NHL)],
                                        qq, dt, b))
                    return sh_done

                for h in range(NHL):
                    pav0 = psB.tile([P, QC], F32, tag="pav0", bufs=2,
                                    name="pav0")
                    pav1 = psB.tile([P, QC], F32, tag="pav1", bufs=1,
                                    name="pav1")
                    pav = (pav0, pav1)
                    acc0 = accp.tile([P, SH], BF16, tag="acc0", name="acc0")
                    acc1 = accp.tile([P, SH], BF16, tag="acc1", name="acc1")
                    atq = deque()

                    def emit_av(h=h, pav=pav, acc0=acc0, acc1=acc1):
                        kt, a8 = atq.popleft()
                        for jq in (0, 1):
                            nc.tensor.matmul(
                                pav[jq][:, :],
                                lhsT=vres8[:, kt, h * HD:(h + 1) * HD],
                                rhs=a8[:, jq * QC:(jq + 1) * QC],
                                start=(kt == 0), stop=(kt == KO - 1))
                        # rowsum accumulators: Pool takes 4 early/mid
                        # tiles only -- its 2.1us adds must all land
                        # before the next head's prs matmuls read acc1
                        if kt in (1, 4, 7, 10):
                            eng, acct = nc.gpsimd, acc1
                            first = kt == 1
                        else:
                            eng, acct = nc.vector, acc0
                            first = kt == 0
                        if first:
                            eng.tensor_copy(acct[:], a8[:])
                        else:
                            eng.tensor_add(acct[:], acct[:], a8[:])

                    for kt in range(KO):
                        at8 = atp.tile([P, SH], BF16, tag="at8",
                                       name="at8")
                        for jq in (0, 1):
                            psc = psB.tile([P, QC], F32, tag="psc",
                                           bufs=3, name="psc")
                            nc.tensor.matmul(
                                psc[:, :],
                                lhsT=kres[h][:, kt * P:(kt + 1) * P],
                                rhs=qres[h][:, q0 + jq * QC:
                                            q0 + (jq + 1) * QC],
                                start=True, stop=True)
                            nc.scalar.activation(
                                at8[:, jq * QC:(jq + 1) * QC], psc[:, :],
                                func=Act.Exp, bias=ebias[:], scale=SCALE)
                        atq.append((kt, at8))
                        if len(atq) >= 3:
                            emit_av()
                        if kt in (0, 4) and pend_nm:
                            pend_nm.popleft()()
                        if kt in (2, 6) and pend_na:
                            pend_na.popleft()()
                        # the kt0 po-group is Scalar-independent PE
                        # filler that paces psc issue while Scalar
                        # drains its head-start exp backlog
                        if pend_o and kt in (0, 3, 5, 8, 9, 11,
                                             13, 15):
                            pend_o.popleft()()
                    while atq:
                        emit_av()
                    done = make_sh_done() if h == NHL - 1 else None
                    prs_store = {}
                    pend_nm.append(make_norm_mm(acc0, acc1, prs_store, 1,
                                                pav))
                    pend_nm.append(make_norm_mm(acc0, acc1, prs_store, 0))
                    pend_na.append(make_norm_act(h, pav, prs_store, avn,
                                                 done, 1))
                    pend_na.append(make_norm_act(h, pav, prs_store, avn,
                                                 None, 0))
            while pend_nm:
                pend_nm.popleft()()
            while pend_na:
                pend_na.popleft()()
                # the 16 jq1 oproj groups unblock after act1; drain
                # them while Scalar runs the jq0 normalize chain
                for _ in range(16):
                    if pend_o:
                        pend_o.popleft()()
            while pend_o:
                pend_o.popleft()()


def _build_program(loop_n=0, phases=(1, 2)):
    key = ("nc", loop_n, tuple(phases))
    if key in _PROG_CACHE:
        return _PROG_CACHE[key]
    nc = bass.Bass()
    t = {}
    t["wq16"] = nc.dram_tensor("wq16", [P, KO, IL], F16, kind="ExternalInput")
    t["wk16"] = nc.dram_tensor("wk16", [P, KO, IL], F16, kind="ExternalInput")
    t["wv16"] = nc.dram_tensor("wv16", [P, KO, IL], F16, kind="ExternalInput")
    t["x16"] = nc.dram_tensor("x16", [NSC, 2, P, KO, SC2], F16,
                              kind="ExternalInput")
    t["woT"] = nc.dram_tensor("woT", [P, NHL, D], BF16, kind="ExternalInput")
    for nm in ("ropeAq", "ropeBq", "ropeAk", "ropeBk"):
        t[nm] = nc.dram_tensor(nm, [P, S], F16, kind="ExternalInput")
    t["outT"] = nc.dram_tensor("outT", [D, S], BF16, kind="ExternalOutput")
    if loop_n == -1:  # debug taps build
        t["dbg_qkv"] = nc.dram_tensor("dbg_qkv", [2, NHL, P, S], F16,
                                      kind="ExternalOutput")
        t["dbg_v"] = nc.dram_tensor("dbg_v", [P, KO, IL], BF16,
                                    kind="ExternalOutput")
        t["dbg_avn"] = nc.dram_tensor("dbg_avn", [NHL, P, QC], BF16,
                                      kind="ExternalOutput")

    with tile.TileContext(nc) as tc:
        _emit(nc, tc, t, phases)
    _split_multi_waits(nc)
    _PROG_CACHE[key] = nc
    return nc


def _prep_in_maps(x, rope_emb, Wq, Wk, Wv, Wo, q_norm_w, k_norm_w):
    x = np.asarray(x, np.float32)
    F = np.asarray(rope_emb, np.float32)[:, 0]          # [S, 64, 2, 2]
    A0 = np.concatenate([F[:, :, 0, 0], F[:, :, 1, 1]], axis=-1)  # [S, 128]
    B0 = np.concatenate([F[:, :, 0, 1], F[:, :, 1, 0]], axis=-1)  # [S, 128]

    def rope_consts(w):
        w = np.asarray(w, np.float32)
        w_sw = np.concatenate([w[64:], w[:64]])
        A = np.ascontiguousarray((A0 * w[None, :]).T).astype(np.float16)
        B = np.ascontiguousarray((B0 * w_sw[None, :]).T).astype(np.float16)
        return A, B

    Aq, Bq = rope_consts(q_norm_w)
    Ak, Bk = rope_consts(k_norm_w)

    def to_dev(arr, kt, width, dtype):
        # [D_like, width] -> [128, kt, width] with row index = kt*128 + p
        return np.ascontiguousarray(
            arr.reshape(kt, P, width).transpose(1, 0, 2)).astype(dtype)

    Wq = np.asarray(Wq, np.float32)
    Wk = np.asarray(Wk, np.float32)
    Wv = np.asarray(Wv, np.float32)
    Wo = np.asarray(Wo, np.float32)

    in_maps = []
    for c in range(N_CORES):
        b, hg = divmod(c, NH // NHL)
        sl = slice(hg * IL, (hg + 1) * IL)
        in_maps.append({
            "wq16": to_dev(np.ascontiguousarray(Wq[sl, :].T), KO, IL,
                           np.float16),
            "wk16": to_dev(np.ascontiguousarray(Wk[sl, :].T), KO, IL,
                           np.float16),
            "wv16": to_dev(np.ascontiguousarray(Wv[sl, :].T), KO, IL,
                           np.float16),
            "x16": np.ascontiguousarray(
                x[b].T.reshape(KO, P, NSC, 2, SC2).transpose(2, 3, 1, 0, 4)
            ).astype(np.float16),
            "woT": to_dev(np.ascontiguousarray(Wo[:, sl].T), NHL, D,
                          ml_dtypes.bfloat16),
            "ropeAq": Aq, "ropeBq": Bq, "ropeAk": Ak, "ropeBk": Bk,
        })
    return in_maps


def kernel(x, rope_emb, Wq, Wk, Wv, Wo, q_norm_w, k_norm_w, _trace=False):
    nc = _build_program()
    in_maps = _prep_in_maps(x, rope_emb, Wq, Wk, Wv, Wo, q_norm_w, k_norm_w)
    res = run_bass_kernel_spmd(nc, in_maps, core_ids=list(range(N_CORES)),
                               trace=_trace)
    out = np.empty((2, S, D), np.float32)
    for b in range(2):
        acc = res.results[4 * b]["outT"].astype(np.float32)
        for hg in range(1, 4):
            acc += res.results[4 * b + hg]["outT"].astype(np.float32)
        out[b] = acc.T
    if _trace:
        kernel.last_exec_time_ns = res.exec_time_ns
        kernel.last_results = res
    return out



# revision 71
# speedup vs baseline: 1.0134x; 1.0007x over previous
"""DiT attention block on 8 Trainium2 NeuronCores.

Sharding: batch (2) x head-groups (4 heads each) -> 8 cores.  Each core
computes q/k/v projections, RMSNorm+rope on q/k, softmax attention, and
its partial output projection for its 4 heads; the host sums the 4
head-group bf16 partials per batch and transposes back.

Implementation notes (v4, all matmuls fp16/bf16 -- fp8 was measured and
rejected: its ~3% weight noise lands unaveraged on near-one-hot softmax
columns and blows the 2e-2 gate):
  - Startup: every bulk tensor is halved across the three DMA queues
    (sync/scalar HW-DGE + gpsimd SW-DGE) in first-needed-first ring
    order, so the first V matmul fires at ~20us instead of ~35us.  x
    chunks are half-major [P, 2, KO, SC2] so each half lands as one
    contiguous 8KB/partition transfer (a strided half fragments into
    512B packets at ~50GB/s).  Chunk halves for sc>=1 ride scalar+sync;
    gpsimd's SW-DGE prep would execute late behind its stage-A adds.
  - Stage A streams x in 4 chunks; K/Q are produced transposed per head
    ([head_dim, seq]); RMSNorm sum-of-squares is ONE ones[128,128]
    matmul per head that reduces AND broadcasts across partitions
    (saves the old separate per-head rstd-broadcast matmul), rstd =
    exp(-ln(.)/2) on ACT, rope rotate-half is a partition-offset
    SBUF->SBUF DMA.  Per-head tails are software-pipelined ~2 heads
    late; the last chunk's Q tails do their rope add on DVE because the
    stage-boundary drain serializes on GpSimd's 1.15us adds otherwise.
  - Stage B processes seq in halves: per (head, kt) fp16 scores ->
    exp(scale*s - 2) into bf16 "at" tiles; A@V accumulates in PSUM;
    rowsums accumulate on DVE (12/16, incl. the late kts) + GpSimd
    (only kts 1/4/7/10 -- its 2.1us adds must finish before the next
    head's rowsum matmuls read acc1).  The normalize chain is split and
    staggered into the next head's slots: rowsum matmuls at kt0/kt4,
    Ln+Exp+mul at kt2/kt6, so the Ln never head-of-line-blocks Scalar's
    exp FIFO (that convoy stalled psc recycling 2.6us/head and
    re-throttled HAM to 1.2GHz).  The single-buffer pav1 bank is
    evacuated to SBUF (bf16) by a Vector copy at kt0 so the next head's
    first A@V write never waits on the normalize chain.  Output-
    projection dt-groups from the previous half interleave at kts
    0,3,5,8,9,11,13,15 -- the kt0 group is Scalar-independent filler
    that paces psc issue against the head-start exp backlog; the rest
    stay clear of the prs bank-rotation slots.
  - Known ceilings: stage B is co-limited by Scalar (32 exps + 4 norm
    ACTs ~= 21us/head vs PE 21.3us/head); the ~10us NEFF teardown and
    ~9us DMA-ring spin-up at start are fixed costs.
"""

import math

import ml_dtypes
import numpy as np

import concourse.bass as bass
import concourse.mybir as mybir
import concourse.tile as tile
from concourse.bass_utils import run_bass_kernel_spmd

F32 = mybir.dt.float32
F16 = mybir.dt.float16
BF16 = mybir.dt.bfloat16
F8 = mybir.dt.float8e4
F8E5 = mybir.dt.float8e5
E4 = ml_dtypes.float8_e4m3
DR = mybir.MatmulPerfMode.DoubleRow
Act = mybir.ActivationFunctionType

P = 128          # partitions / head_dim
S = 2048         # sequence
D = 2048         # model dim
HD = 128         # head dim
NH = 16          # total heads
NHL = 4          # heads per core
IL = NHL * HD    # 512, inner slice per core
KO = D // P      # 16 contraction subtiles
KO2 = KO // 2    # 8 DoubleRow pairs
SC = 512         # x-chunk columns (projection phase)
SC2 = SC // 2    # token-half split of a chunk's DMA
NSC = S // SC    # 4
QC = 512         # q-chunk columns (attention phase)
NQC = S // QC    # 4
EPS = 1e-6
SCALE = 1.0 / math.sqrt(HD)
WS = math.sqrt(D)              # host-side weight pre-scale
SSQ_SCALE = 1.0 / HD          # with WS^2*EPS bias: rstd comes out as rstd_true/WS
EXP_BIAS = -2.0   # cancels in softmax; keeps exp moderate
N_CORES = 8

_PROG_CACHE = {}


def _split_multi_waits(nc, max_waits=1):
    """walrus here rejects >1 sync-wait per instruction; move extras onto
    same-engine nops placed immediately before the instruction."""
    n_split = 0
    for fn in nc.m.functions:
        for bb in fn.blocks:
            insts = bb.instructions
            new_list = []
            changed = False
            for inst in insts:
                si = getattr(inst, "sync_info", None)
                waits = list(si.on_wait) if (si is not None and si.on_wait) else []
                if len(waits) > max_waits:
                    extra = waits[:-max_waits]
                    keep = waits[-max_waits:]
                    for i in range(0, len(extra), max_waits):
                        nop = mybir.InstNoOp(
                            name=f"I-wsplit-{nc.next_id()}", ins=[], outs=[])
                        nop.engine = inst.engine
                        nop.sync_info = mybir.SyncInfo(
                            on_wait=extra[i:i + max_waits], on_update=[])
                        new_list.append(nop)
                        n_split += 1
                    del si.on_wait[:]
                    si.on_wait.extend(keep)
                    changed = True
                new_list.append(inst)
            if changed:
                del insts[:]
                insts.extend(new_list)
    return n_split


def _emit(nc, tc, t, phases=(1, 2)):
    from contextlib import ExitStack

    with ExitStack() as top:
        top.enter_context(nc.allow_low_precision(
            reason="fp8 DoubleRow matmuls; fp32 kept where it matters"))
        const = top.enter_context(tc.tile_pool(name="const", bufs=1))

        rope = {}
        for nm in ("ropeAq", "ropeBq", "ropeAk", "ropeBk"):
            til = const.tile([P, S], F16, tag=nm, name=nm)
            rope[nm] = til
        onesb = const.tile([P, P], BF16, tag="onesb")
        nc.vector.memset(onesb, 1.0)
        eps_t = const.tile([P, 1], F32, tag="eps")
        nc.vector.memset(eps_t, EPS)
        ebias = const.tile([P, 1], F32, tag="ebias")
        nc.vector.memset(ebias, EXP_BIAS)

        resid = top.enter_context(tc.tile_pool(name="resid", bufs=1))
        w8 = {}
        for nm in ("wv16", "wk16", "wq16"):
            w8[nm] = resid.tile([P, KO, IL], F16, tag=nm, name=nm)
        # DMA ring order is first-needed-first; each weight is halved
        # across the two bulk queues (scalar HW-DGE + gpsimd SW-DGE) so
        # the V projection can start ~12us earlier.  sync carries the
        # chunk-0 x halves then stays latency-clean for psw swaps.
        # wv is split 9/7: gpsimd's ring starts ~1us later and runs
        # ~10% slower, so equalizing completion gates the first matmul
        # ~2.5us earlier than an even split
        nc.scalar.dma_start(w8["wv16"][:, 0:9, :], t["wv16"][:, 0:9, :])
        nc.gpsimd.dma_start(w8["wv16"][:, 9:KO, :], t["wv16"][:, 9:KO, :])
        for nm in ("wk16", "wq16"):
            nc.scalar.dma_start(w8[nm][:, 0:KO2, :], t[nm][:, 0:KO2, :])
            nc.gpsimd.dma_start(w8[nm][:, KO2:KO, :], t[nm][:, KO2:KO, :])
        nc.scalar.dma_start(rope["ropeAq"][:], t["ropeAq"][:, :])
        nc.scalar.dma_start(rope["ropeBq"][:], t["ropeBq"][:, :])

        qres = [resid.tile([P, S], F16, tag=f"qres{h}", name=f"qres{h}")
                for h in range(NHL)]
        kres = [resid.tile([P, S], F16, tag=f"kres{h}", name=f"kres{h}")
                for h in range(NHL)]
        vres8 = resid.tile([P, KO, IL], BF16, tag="vres8")

        # ---------------- stage A: projections + RMSNorm + rope -------------
        if 1 in phases:
          with ExitStack() as ph:
            sa = ph.enter_context(tc.tile_pool(name="sa", bufs=3))
            xqp = ph.enter_context(tc.tile_pool(name="xqp", bufs=2))
            nrm = ph.enter_context(tc.tile_pool(name="nrm", bufs=2))
            psA = ph.enter_context(tc.tile_pool(name="psA", bufs=1, space="PSUM"))

            from collections import deque
            pend = deque()      # deferred small PE-tail closures

            def pop_pend(n, lag=2):
                # keep `lag` closures queued so tails run ~2 units late
                for _ in range(n):
                    if len(pend) > lag:
                        pend.popleft()()

            def emit_head(wname, ra, rb, dst, s0, xq, h, tail_eng=None):
                """fp16 projection for one head; defer ssq/psw/rope tail."""
                praw = psA.tile([P, SC], F32, tag="praw", bufs=2,
                                name="praw")
                for kk in range(KO):
                    nc.tensor.matmul(
                        praw[:, :],
                        lhsT=w8[wname][:, kk, h * P:(h + 1) * P],
                        rhs=xq[:, :, kk, :],
                        start=(kk == 0), stop=(kk == KO - 1))
                raw = sa.tile([P, SC], F16, tag="raw", name="raw")
                nc.scalar.copy(raw[:], praw[:, :])
                sq = sa.tile([P, SC], BF16, tag="sq", name="sq")
                nc.scalar.activation(sq[:], praw[:, :], func=Act.Square)

                def tail():
                    # sum-of-squares reduced AND broadcast across all
                    # 128 partitions in one ones[128,128] matmul (the
                    # old [1,SC] reduce needed a second broadcast
                    # matmul per head; this saves 512 PE cols/head)
                    pbq = psA.tile([P, SC], F32, tag="pssq", bufs=4,
                                   name="pbq")
                    nc.tensor.matmul(pbq[:, :], lhsT=onesb[:, :],
                                     rhs=sq[:], start=True, stop=True)
                    psw = sa.tile([P, SC], F16, tag="psw", name="psw")
                    nc.sync.dma_start(psw[0:64, :], raw[64:128, :])
                    nc.sync.dma_start(psw[64:128, :], raw[0:64, :])
                    tmp = sa.tile([P, SC], F16, tag="tmp", name="tmp")
                    nc.vector.tensor_mul(tmp[:], ra[:, s0:s0 + SC], raw[:])
                    tmp2 = sa.tile([P, SC], F16, tag="tmp2", name="tmp2")
                    nc.vector.tensor_mul(tmp2[:], rb[:, s0:s0 + SC],
                                         psw[:, :])
                    t3 = sa.tile([P, SC], F16, tag="t3", name=f"t3_{h}",
                                 bufs=6)
                    (tail_eng or nc.gpsimd).tensor_add(t3[:], tmp[:],
                                                       tmp2[:])

                    def tail2():
                        lnv = nrm.tile([P, SC], F32, tag="lnv",
                                       name="lnv")
                        nc.scalar.activation(lnv[:], pbq[:, :],
                                             func=Act.Ln, bias=eps_t[:],
                                             scale=SSQ_SCALE)
                        rstd = nrm.tile([P, SC], F16, tag="rstd",
                                        name="rstd")
                        nc.scalar.activation(rstd[:], lnv[:],
                                             func=Act.Exp, scale=-0.5)
                        nc.vector.tensor_mul(
                            dst[h][:, s0:s0 + SC], t3[:], rstd[:])
                    pend.append(tail2)
                pend.append(tail)

            for sc in range(NSC):
                s0 = sc * SC
                # V projection ([s, il] layout; x subtiles stationary).
                # xq is half-major [P, 2, KO, SC2] so each DMA half is a
                # contiguous 8KB/partition transfer (strided halves
                # fragmented into 512B packets and ran at ~50GB/s).
                xq = xqp.tile([P, 2, KO, SC2], F16, tag="xq", name="xq")
                if sc == 0:
                    nc.sync.dma_start(xq[:, 0], t["x16"][sc, 0, :, :, :])
                    nc.sync.dma_start(xq[:, 1], t["x16"][sc, 1, :, :, :])
                    # K-rope consts ride sync after the chunk-0 halves
                    nc.sync.dma_start(rope["ropeAk"][:], t["ropeAk"][:, :])
                    nc.sync.dma_start(rope["ropeBk"][:], t["ropeBk"][:, :])
                else:
                    nc.scalar.dma_start(xq[:, 0], t["x16"][sc, 0, :, :, :])
                    nc.sync.dma_start(xq[:, 1], t["x16"][sc, 1, :, :, :])
                for st in range(SC // P):
                    hf, ho = st // 2, (st % 2) * P
                    pv = psA.tile([P, SC], F32, tag="pv", bufs=2, name="pv")
                    for kk in range(KO):
                        nc.tensor.matmul(
                            pv[:, :],
                            lhsT=xq[:, hf, kk, ho:ho + P],
                            rhs=w8["wv16"][:, kk, :],
                            start=(kk == 0), stop=(kk == KO - 1))
                    nc.vector.tensor_copy(vres8[:, sc * (SC // P) + st, :],
                                          pv[:, :])
                    pop_pend(2)

                # K then Q projections (transposed per head) + norm + rope
                for wname, ra, rb, dst in (
                    ("wk16", rope["ropeAk"], rope["ropeBk"], kres),
                    ("wq16", rope["ropeAq"], rope["ropeBq"], qres),
                ):
                    # last chunk's Q tails drain serially at the stage
                    # boundary; Vector's 426ns adds beat GpSimd's 1.15us
                    teng = (nc.vector if (sc == NSC - 1
                                          and wname == "wq16") else None)
                    for h in range(NHL):
                        emit_head(wname, ra, rb, dst, s0, xq, h, teng)
                        pop_pend(2)
            while pend:
                pend.popleft()()

        if "dbg_qkv" in t:
            for h in range(NHL):
                nc.sync.dma_start(t["dbg_qkv"][0, h, :, :], qres[h][:])
                nc.sync.dma_start(t["dbg_qkv"][1, h, :, :], kres[h][:])
        if "dbg_v" in t:
            nc.sync.dma_start(t["dbg_v"][:, :, :], vres8[:])

        # ---------------- stage B: attention + output projection ------------
        if 2 in phases:
          with ExitStack() as ph:
            atp = ph.enter_context(tc.tile_pool(name="atp", bufs=5))
            wop = ph.enter_context(tc.tile_pool(name="wop", bufs=1))
            wo = wop.tile([P, NHL, D], BF16, tag="wo", name="wo")
            nc.gpsimd.dma_start(wo[:], t["woT"][:, :, :])
            avp = ph.enter_context(tc.tile_pool(name="avp", bufs=5))
            rcpp = ph.enter_context(tc.tile_pool(name="rcpp", bufs=2))
            outs = ph.enter_context(tc.tile_pool(name="outs", bufs=4))
            psB = ph.enter_context(tc.tile_pool(name="psB", bufs=1, space="PSUM"))

            from collections import deque
            pend_o = deque()    # out-proj dt-group closures from prev qc

            def emit_oproj(avn, q0, dt, burst=False):
                po = psB.tile([P, QC], F32, tag="po", bufs=2, name="po")
                for it in range(NHL):
                    nc.tensor.matmul(
                        po[:, :],
                        lhsT=wo[:, it, dt * P:(dt + 1) * P],
                        rhs=avn[it][:],
                        start=(it == 0), stop=(it == NHL - 1))
                osb = outs.tile([P, QC], BF16, tag="osb", name="osb")
                if burst:
                    # in the closing oproj burst Scalar has no exps
                    # left; putting the copies there frees Vector and
                    # decouples the po bank rotation from its queue
                    nc.scalar.copy(osb[:], po[:, :])
                else:
                    nc.vector.tensor_copy(osb[:], po[:, :])
                nc.sync.dma_start(
                    t["outT"][dt * P:(dt + 1) * P, q0:q0 + QC], osb[:])

            SH = 2 * QC          # q-half width (1024)
            accp = ph.enter_context(tc.tile_pool(name="accp", bufs=3))
            pend_nm = deque()    # deferred rowsum matmuls (per head)
            pend_na = deque()    # deferred normalize ACT chains (per head)

            def make_norm_mm(acc0, acc1, prs_store, jq, pav=None):
                # rowsum matmuls run ~2 slots ahead of the ACT chain so
                # the Ln never waits on them inside Scalar's FIFO
                # (head-of-line blocking there stalled psc recycling).
                # jq1 first: its chain releases the single-buffer pav1.
                def norm_mm():
                    if pav is not None:
                        # evacuate single-buffer pav1 to SBUF right at
                        # kt0 so the next head's first A@V write never
                        # waits for the normalize chain (which sits
                        # behind Scalar's head-start exp backlog)
                        pavs = avp.tile([P, QC], BF16, tag="pavs",
                                        name="pavs", bufs=2)
                        nc.vector.tensor_copy(pavs[:], pav[jq][:, :])
                        prs_store["pavs"] = pavs
                    prs = psB.tile([P, QC], F32, tag="po", bufs=2,
                                   name="prs")
                    nc.tensor.matmul(
                        prs[:, :], lhsT=onesb[:, :],
                        rhs=acc0[:, jq * QC:(jq + 1) * QC],
                        start=True, stop=False)
                    nc.tensor.matmul(
                        prs[:, :], lhsT=onesb[:, :],
                        rhs=acc1[:, jq * QC:(jq + 1) * QC],
                        start=False, stop=True)
                    prs_store[jq] = prs
                return norm_mm

            def make_norm_act(h, pav, prs_store, avn, sh_done, jq,
                              use_recip=False):
                def norm_act():
                    if use_recip:
                        # sh0 has no oproj interleave: PE ~15.4us/head
                        # vs Scalar ~21us -> 1/rowsum goes to DVE
                        # (half-idle in sh0).  sh1 is PE-bound, so the
                        # chains popping there keep Scalar's Ln+Exp.
                        rcpb = rcpp.tile([P, QC], F32, tag="lnp",
                                         name="rcp")
                        nc.vector.reciprocal(rcpb[:],
                                             prs_store[jq][:, :])
                    else:
                        lnp = rcpp.tile([P, QC], F32, tag="lnp",
                                        name="lnp")
                        nc.scalar.activation(lnp[:], prs_store[jq][:, :],
                                             func=Act.Ln)
                        rcpb = rcpp.tile([P, QC], BF16, tag="rcpb",
                                         name="rcpb")
                        nc.scalar.activation(rcpb[:], lnp[:],
                                             func=Act.Exp, scale=-1.0)
                    an = avp.tile([P, QC], BF16, tag="avn",
                                  name=f"an{h}_{jq}", bufs=16)
                    src = (prs_store["pavs"] if "pavs" in prs_store
                           and jq == 1 else pav[jq])
                    nc.vector.tensor_mul(an[:], src[:, :], rcpb[:])
                    avn[(h, jq)] = an
                    if sh_done is not None:
                        sh_done()
                return norm_act

            for sh in range(S // SH):
                q0 = sh * SH
                avn = {}         # (h, jq) -> normalized attention out

                def make_sh_done(avn=avn, q0=q0):
                    def sh_done():
                        if "dbg_avn" in t and q0 == 0:
                            for hh in range(NHL):
                                nc.sync.dma_start(t["dbg_avn"][hh, :, :],
                                                  avn[(hh, 0)][:])
                        # jq1 first: at the final drain these groups
                        # unblock after act1 alone, so they overlap the
                        # jq0 normalize chains
                        burst = q0 == S - SH
                        for jq in (1, 0):
                            for dt in range(D // P):
                                # avn looked up lazily: jq0 tiles don't
                                # exist yet when sh_done fires on act1
                                pend_o.append(
                                    lambda jq=jq, qq=q0 + jq * QC,
                                    dt=dt, b=burst: emit_oproj(
                                        [avn[(it, jq)]
                                         for it in range(NHL)],
                                        qq, dt, b))
                    return sh_done

                for h in range(NHL):
                    pav0 = psB.tile([P, QC], F32, tag="pav0", bufs=2,
                                    name="pav0")
                    pav1 = psB.tile([P, QC], F32, tag="pav1", bufs=1,
                                    name="pav1")
                    pav = (pav0, pav1)
                    acc0 = accp.tile([P, SH], BF16, tag="acc0", name="acc0")
                    acc1 = accp.tile([P, SH], BF16, tag="acc1", name="acc1")
                    atq = deque()

                    def emit_av(h=h, pav=pav, acc0=acc0, acc1=acc1):
                        kt, a8 = atq.popleft()
                        for jq in (0, 1):
                            nc.tensor.matmul(
                                pav[jq][:, :],
                                lhsT=vres8[:, kt, h * HD:(h + 1) * HD],
                                rhs=a8[:, jq * QC:(jq + 1) * QC],
                                start=(kt == 0), stop=(kt == KO - 1))
                        # rowsum accumulators: Pool takes 4 early/mid
                        # tiles only -- its 2.1us adds must all land
                        # before the next head's prs matmuls read acc1
                        if kt in (1, 4, 7, 10):
                            eng, acct = nc.gpsimd, acc1
                            first = kt == 1
                        else:
                            eng, acct = nc.vector, acc0
                            first = kt == 0
                        if first:
                            eng.tensor_copy(acct[:], a8[:])
                        else:
                            eng.tensor_add(acct[:], acct[:], a8[:])

                    for kt in range(KO):
                        at8 = atp.tile([P, SH], BF16, tag="at8",
                                       name="at8")
                        for jq in (0, 1):
                            psc = psB.tile([P, QC], F32, tag="psc",
                                           bufs=3, name="psc")
                            nc.tensor.matmul(
                                psc[:, :],
                                lhsT=kres[h][:, kt * P:(kt + 1) * P],
                                rhs=qres[h][:, q0 + jq * QC:
                                            q0 + (jq + 1) * QC],
                                start=True, stop=True)
                            nc.scalar.activation(
                                at8[:, jq * QC:(jq + 1) * QC], psc[:, :],
                                func=Act.Exp, bias=ebias[:], scale=SCALE)
                        atq.append((kt, at8))
                        if len(atq) >= 3:
                            emit_av()
                        if kt in (0, 4) and pend_nm:
                            pend_nm.popleft()()
                        if kt in (2, 6) and pend_na:
                            pend_na.popleft()()
                        # the kt0 po-group is Scalar-independent PE
                        # filler that paces psc issue while Scalar
                        # drains its head-start exp backlog
                        if pend_o and kt in (0, 3, 5, 8, 9, 11,
                                             13, 15):
                            pend_o.popleft()()
                    while atq:
                        emit_av()
                    done = make_sh_done() if h == NHL - 1 else None
                    prs_store = {}
                    pend_nm.append(make_norm_mm(acc0, acc1, prs_store, 1,
                                                pav))
                    pend_nm.append(make_norm_mm(acc0, acc1, prs_store, 0))
                    ur = sh == 0 and h < NHL - 1
                    pend_na.append(make_norm_act(h, pav, prs_store, avn,
                                                 done, 1, ur))
                    pend_na.append(make_norm_act(h, pav, prs_store, avn,
                                                 None, 0, ur))
            while pend_nm:
                pend_nm.popleft()()
            while pend_na:
                pend_na.popleft()()
                # the 16 jq1 oproj groups unblock after act1; drain
                # them while Scalar runs the jq0 normalize chain
                for _ in range(16):
                    if pend_o:
                        pend_o.popleft()()
            while pend_o:
                pend_o.popleft()()


def _build_program(loop_n=0, phases=(1, 2)):
    key = ("nc", loop_n, tuple(phases))
    if key in _PROG_CACHE:
        return _PROG_CACHE[key]
    nc = bass.Bass()
    t = {}
    t["wq16"] = nc.dram_tensor("wq16", [P, KO, IL], F16, kind="ExternalInput")
    t["wk16"] = nc.dram_tensor("wk16", [P, KO, IL], F16, kind="ExternalInput")
    t["wv16"] = nc.dram_tensor("wv16", [P, KO, IL], F16, kind="ExternalInput")
    t["x16"] = nc.dram_tensor("x16", [NSC, 2, P, KO, SC2], F16,
                              kind="ExternalInput")
    t["woT"] = nc.dram_tensor("woT", [P, NHL, D], BF16, kind="ExternalInput")
    for nm in ("ropeAq", "ropeBq", "ropeAk", "ropeBk"):
        t[nm] = nc.dram_tensor(nm, [P, S], F16, kind="ExternalInput")
    t["outT"] = nc.dram_tensor("outT", [D, S], BF16, kind="ExternalOutput")
    if loop_n == -1:  # debug taps build
        t["dbg_qkv"] = nc.dram_tensor("dbg_qkv", [2, NHL, P, S], F16,
                                      kind="ExternalOutput")
        t["dbg_v"] = nc.dram_tensor("dbg_v", [P, KO, IL], BF16,
                                    kind="ExternalOutput")
        t["dbg_avn"] = nc.dram_tensor("dbg_avn", [NHL, P, QC], BF16,
                                      kind="ExternalOutput")

    with tile.TileContext(nc) as tc:
        _emit(nc, tc, t, phases)
    _split_multi_waits(nc)
    _PROG_CACHE[key] = nc
    return nc


def _prep_in_maps(x, rope_emb, Wq, Wk, Wv, Wo, q_norm_w, k_norm_w):
    x = np.asarray(x, np.float32)
    F = np.asarray(rope_emb, np.float32)[:, 0]          # [S, 64, 2, 2]
    A0 = np.concatenate([F[:, :, 0, 0], F[:, :, 1, 1]], axis=-1)  # [S, 128]
    B0 = np.concatenate([F[:, :, 0, 1], F[:, :, 1, 0]], axis=-1)  # [S, 128]

    def rope_consts(w):
        w = np.asarray(w, np.float32)
        w_sw = np.concatenate([w[64:], w[:64]])
        A = np.ascontiguousarray((A0 * w[None, :]).T).astype(np.float16)
        B = np.ascontiguousarray((B0 * w_sw[None, :]).T).astype(np.float16)
        return A, B

    Aq, Bq = rope_consts(q_norm_w)
    Ak, Bk = rope_consts(k_norm_w)

    def to_dev(arr, kt, width, dtype):
        # [D_like, width] -> [128, kt, width] with row index = kt*128 + p
        return np.ascontiguousarray(
            arr.reshape(kt, P, width).transpose(1, 0, 2)).astype(dtype)

    Wq = np.asarray(Wq, np.float32)
    Wk = np.asarray(Wk, np.float32)
    Wv = np.asarray(Wv, np.float32)
    Wo = np.asarray(Wo, np.float32)

    in_maps = []
    for c in range(N_CORES):
        b, hg = divmod(c, NH // NHL)
        sl = slice(hg * IL, (hg + 1) * IL)
        in_maps.append({
            "wq16": to_dev(np.ascontiguousarray(Wq[sl, :].T), KO, IL,
                           np.float16),
            "wk16": to_dev(np.ascontiguousarray(Wk[sl, :].T), KO, IL,
                           np.float16),
            "wv16": to_dev(np.ascontiguousarray(Wv[sl, :].T), KO, IL,
                           np.float16),
            "x16": np.ascontiguousarray(
                x[b].T.reshape(KO, P, NSC, 2, SC2).transpose(2, 3, 1, 0, 4)
            ).astype(np.float16),
            "woT": to_dev(np.ascontiguousarray(Wo[:, sl].T), NHL, D,
                          ml_dtypes.bfloat16),
            "ropeAq": Aq, "ropeBq": Bq, "ropeAk": Ak, "ropeBk": Bk,
        })
    return in_maps


def kernel(x, rope_emb, Wq, Wk, Wv, Wo, q_norm_w, k_norm_w, _trace=False):
    nc = _build_program()
    in_maps = _prep_in_maps(x, rope_emb, Wq, Wk, Wv, Wo, q_norm_w, k_norm_w)
    res = run_bass_kernel_spmd(nc, in_maps, core_ids=list(range(N_CORES)),
                               trace=_trace)
    out = np.empty((2, S, D), np.float32)
    for b in range(2):
        acc = res.results[4 * b]["outT"].astype(np.float32)
        for hg in range(1, 4):
            acc += res.results[4 * b + hg]["outT"].astype(np.float32)
        out[b] = acc.T
    if _trace:
        kernel.last_exec_time_ns = res.exec_time_ns
        kernel.last_results = res
    return out

